# revision 1
# baseline (speedup 1.0000x reference)
"""DGCNN (4x EdgeConv + final projection + global max) on 8 Trainium2 cores.

Sharding: data-parallel over batch B=8 -> one point cloud per NeuronCore.

Per-core algorithm (N=2048 points, k=40 neighbors):
  Each EdgeConv layer `h' = max_k lrelu(concat(h_j - h_i, h_i) @ W + b)` is
  algebraically decomposed (lrelu monotone, V_i constant over neighbors j):
      U = h @ W_top          (N, D)
      V = h @ (W_bot - W_top) + b
      h'[i] = lrelu(max_{j in knn(i)} U[j] + V[i])
  so no (N, k, 2C) edge tensor is ever materialized.

  kNN: top-40 of score T[i,j] = 2 h_i.h_j - |h_j|^2 (row-equivalent ordering
  to -dist). T computed by the PE with augmented operands, top-40 extracted
  per 128-row tile on the DVE via 5 rounds of max8/max_index/match_replace.
  The neighbor-feature max is a DMA gather from U in HBM with the SDMA CCE
  max-accumulate (indirect_dma_start compute_op=max), so the k-reduction
  costs no vector-engine time.
"""

import numpy as np

import concourse.mybir as mybir
import concourse.tile as tile
from concourse import bass_utils
from concourse.bacc import Bacc
from concourse.bass import IndirectOffsetOnAxis
from concourse.masks import make_identity

FP32 = mybir.dt.float32
FP16 = mybir.dt.float16
U32 = mybir.dt.uint32

# Problem constants (hardcoded per harness contract)
B = 8
N = 2048
IN_CHAN = 3
H_DIM = [64, 64, 128, 256]
Z_DIM = 512
K = 40
N_CORES = 8


def build_program(n=N, k=K, in_chan=IN_CHAN, h_dim=None, z2=2 * Z_DIM,
                  group_tiles=4, n_chains=4):
    """Build the single-core Bacc program. Returns (nc, names dict)."""
    h_dim = h_dim or H_DIM
    nt = n // 128            # row tiles
    nfb = n // 512           # 512-wide free blocks
    rounds = k // 8
    dmax = max(h_dim)
    dsum = sum(h_dim)
    n_groups = nt // group_tiles

    nc = Bacc("TRN2", target_bir_lowering=False, debug=False,
              num_devices=N_CORES)

    # ---------------- DRAM tensors ----------------
    xT = nc.dram_tensor("xT", [in_chan, n], FP32, kind="ExternalInput")
    uw_d, vw_d, vb_d = [], [], []
    cins = [in_chan] + [h for h in h_dim[:-1]]
    for l in range(4):
        c, d = cins[l], h_dim[l]
        uw_d.append(nc.dram_tensor(f"uw{l}", [c, d], FP32, kind="ExternalInput"))
        vw_d.append(nc.dram_tensor(f"vw{l}", [c, d], FP32, kind="ExternalInput"))
        vb_d.append(nc.dram_tensor(f"vb{l}", [1, d], FP32, kind="ExternalInput"))
    # final weight chunks: rows split as [h1, h2, h3, h4(128-chunks)]
    wf_chunk_rows = []
    acc = 0
    for l in range(4):
        d = h_dim[l]
        off = 0
        while off < d:
            rows = min(128, d - off)
            wf_chunk_rows.append((l, off, rows, acc))
            acc += rows
            off += rows
    wf_d = [nc.dram_tensor(f"wf{i}", [rows, z2], FP32, kind="ExternalInput")
            for i, (_, _, rows, _) in enumerate(wf_chunk_rows)]

    u_dram = {}
    for l in range(4):
        d = h_dim[l]
        for dc in range((d + 255) // 256):
            dw = min(256, d - dc * 256)
            u_dram[(l, dc)] = nc.dram_tensor(f"u_scratch{l}_{dc}", [n, dw], FP16,
                                             kind="Internal")
    out_dram = nc.dram_tensor("out", [128, z2 // 128], FP32, kind="ExternalOutput")

    with tile.TileContext(nc) as tc:
        with tc.tile_pool(name="pers", bufs=1) as pers, \
             tc.tile_pool(name="sbuf", bufs=2) as sb, \
             tc.tile_pool(name="ps_s", bufs=3, space="PSUM") as ps_s, \
             tc.tile_pool(name="ps_m", bufs=2, space="PSUM") as ps_m:

            # ------------- persistent SBUF -------------
            hT = [pers.tile([max(c, 1), n], FP32, tag=f"hT{l}", name=f"hT{l}")
                  for l, c in enumerate(cins)]
            # layer-3 output (dmax channels) split in 128-row chunks
            h4 = [pers.tile([128, n], FP32, tag=f"h4_{j}", name=f"h4_{j}")
                  for j in range(dmax // 128 if dmax > 128 else 1)]
            ones = pers.tile([1, 128], FP32, tag="ones", name="ones")
            onescol = pers.tile([128, 1], FP32, tag="onescol", name="onescol")
            sqneg = pers.tile([1, n], FP32, tag="sqneg", name="sqneg")
            l2h = pers.tile([128, n], FP32, tag="l2h", name="l2h")
            idx = pers.tile([128, nt, k], U32, tag="idx", name="idx")
            m_sb = pers.tile([128, nt, dmax], FP32, tag="m", name="m")
            v_sb = pers.tile([128, nt, dmax], FP32, tag="v", name="v")
            ident = pers.tile([128, 128], FP32, tag="ident", name="ident")
            uw = [pers.tile([cins[l], h_dim[l]], FP32, tag=f"uw{l}", name=f"uw{l}") for l in range(4)]
            vw = [pers.tile([cins[l], h_dim[l]], FP32, tag=f"vw{l}", name=f"vw{l}") for l in range(4)]
            vb = [pers.tile([1, h_dim[l]], FP32, tag=f"vb{l}", name=f"vb{l}") for l in range(4)]
            wf = [pers.tile([rows, z2], FP32, tag=f"wf{i}", name=f"wf{i}")
                  for i, (_, _, rows, _) in enumerate(wf_chunk_rows)]
            red = pers.tile([128, (z2 // 128) * nfb], FP32, tag="red", name="red")
            out_sb = pers.tile([128, z2 // 128], FP32, tag="out_sb", name="out_sb")

            # ------------- stage inputs -------------
            nc.sync.dma_start(hT[0][:in_chan, :], xT.ap())
            for l in range(4):
                nc.sync.dma_start(uw[l][:], uw_d[l].ap())
                nc.sync.dma_start(vw[l][:], vw_d[l].ap())
                nc.sync.dma_start(vb[l][:], vb_d[l].ap())
            for i in range(len(wf)):
                nc.sync.dma_start(wf[i][:], wf_d[i].ap())
            nc.gpsimd.memset(ones[:], 1.0)
            nc.gpsimd.memset(onescol[:], 1.0)
            make_identity(nc, ident[:])

            amax = mybir.AluOpType.max

            # ------------- EdgeConv layers -------------
            for l in range(4):
                c, d = cins[l], h_dim[l]
                ht = hT[l][:c, :]

                # |h_j|^2 -> sqneg, 2*h -> l2h
                nc.scalar.activation(l2h[:c, :], ht,
                                     mybir.ActivationFunctionType.Square)
                for fb in range(nfb):
                    fs = slice(fb * 512, (fb + 1) * 512)
                    p_sq = ps_m.tile([128, 512], FP32, tag="misc", name="misc")
                    nc.tensor.matmul(p_sq[:1, :], lhsT=onescol[:c, :],
                                     rhs=l2h[:c, fs], start=True, stop=True)
                    nc.scalar.activation(sqneg[:, fs], p_sq[:1, :],
                                         mybir.ActivationFunctionType.Copy,
                                         scale=-1.0)
                nc.scalar.activation(l2h[:c, :], ht,
                                     mybir.ActivationFunctionType.Copy, scale=2.0)

                # U / V for every tile (PE work, independent of top-k)
                for tb in range(nt):
                    bs = slice(tb * 128, (tb + 1) * 128)
                    p_u = ps_m.tile([128, 512], FP32, tag="misc", name="misc")
                    nc.tensor.matmul(p_u[:, :d], lhsT=ht[:, bs], rhs=uw[l][:],
                                     start=True, stop=True)
                    ustage = sb.tile([128, dmax], FP16, tag="ustage", name="ustage")
                    nc.scalar.copy(ustage[:, :d], p_u[:, :d])
                    for dc in range((d + 255) // 256):
                        dw = min(256, d - dc * 256)
                        nc.sync.dma_start(
                            u_dram[(l, dc)].ap().rearrange(
                                "(t p) d -> t p d", p=128)[tb],
                            ustage[:, dc * 256:dc * 256 + dw])

                    p_v = ps_m.tile([128, 512], FP32, tag="misc", name="misc")
                    nc.tensor.matmul(p_v[:, :d], lhsT=ht[:, bs], rhs=vw[l][:],
                                     start=True, stop=False)
                    nc.tensor.matmul(p_v[:, :d], lhsT=ones[:], rhs=vb[l][:],
                                     start=False, stop=True)
                    nc.scalar.copy(v_sb[:, tb, :d], p_v[:, :d])

                # scores + top-k per tile, then gather U rows + k-max
                for tb in range(nt):
                    bs = slice(tb * 128, (tb + 1) * 128)
                    s_sb = sb.tile([128, n], FP32, tag="s_sb", name="s_sb")
                    for fb in range(nfb):
                        fs = slice(fb * 512, (fb + 1) * 512)
                        p_s = ps_s.tile([128, 512], FP32, tag="s", name="s")
                        nc.tensor.matmul(p_s[:], lhsT=l2h[:c, bs],
                                         rhs=ht[:, fs], start=True, stop=False)
                        nc.tensor.matmul(p_s[:], lhsT=ones[:],
                                         rhs=sqneg[:, fs], start=False, stop=True)
                        nc.scalar.copy(s_sb[:, fs], p_s[:])
                    for r in range(rounds):
                        vals8 = sb.tile([128, 8], FP32, tag="vals8", name="vals8")
                        nc.vector.max(out=vals8[:], in_=s_sb[:])
                        nc.vector.max_index(
                            out=idx[:, tb, 8 * r:8 * r + 8],
                            in_max=vals8[:], in_values=s_sb[:])
                        nc.vector.match_replace(
                            out=s_sb[:], in_to_replace=vals8[:],
                            in_values=s_sb[:], imm_value=-3.0e38)
                    # gather the k neighbor U-rows (per 128-channel slab),
                    # then one strided max-reduce over the k axis
                    for dc in range((d + 255) // 256):
                        dw = min(256, d - dc * 256)
                        gdest = sb.tile([128, k, 256], FP16, tag="gdest",
                                        name="gdest")
                        for t in range(k):
                            nc.gpsimd.indirect_dma_start(
                                out=gdest[:, t, :dw],
                                out_offset=None,
                                in_=u_dram[(l, dc)].ap(),
                                in_offset=IndirectOffsetOnAxis(
                                    ap=idx[:, tb, t:t + 1], axis=0),
                                compute_op=mybir.AluOpType.bypass)
                        nc.vector.tensor_reduce(
                            out=m_sb[:, tb, dc * 256:dc * 256 + dw],
                            in_=gdest[:, :, :dw].rearrange("p k d -> p d k"),
                            axis=mybir.AxisListType.X, op=amax)

                # h' = lrelu(M + V) (in place in m_sb; v_sb reused as scratch)
                nc.vector.tensor_tensor(out=m_sb[:, :, :d], in0=m_sb[:, :, :d],
                                        in1=v_sb[:, :, :d], op=mybir.AluOpType.add)
                nc.vector.tensor_scalar_mul(v_sb[:, :, :d], m_sb[:, :, :d], 0.2)
                nc.vector.tensor_tensor(out=m_sb[:, :, :d], in0=m_sb[:, :, :d],
                                        in1=v_sb[:, :, :d], op=amax)

                # transpose h' (N, d) -> next layer's (d, N)
                for tb in range(nt):
                    bs = slice(tb * 128, (tb + 1) * 128)
                    for dc in range((d + 127) // 128):
                        rows = min(128, d - dc * 128)
                        p_t = ps_m.tile([128, 512], FP32, tag="misc", name="misc")
                        nc.tensor.transpose(
                            p_t[:rows, :128],
                            in_=m_sb[:, tb, dc * 128:dc * 128 + rows],
                            identity=ident[:])
                        if l < 3:
                            dst = hT[l + 1][dc * 128:dc * 128 + rows, bs]
                        else:
                            dst = h4[dc][:rows, bs] if d > 128 else h4[0][:rows, bs]
                        nc.scalar.copy(dst, p_t[:rows, :128])

            # ------------- final projection + global max -------------
            h_bufs = {0: hT[1][:h_dim[0], :], 1: hT[2][:h_dim[1], :],
                      2: hT[3][:h_dim[2], :]}
            nmb = z2 // 128
            for mb in range(nmb):
                ms = slice(mb * 128, (mb + 1) * 128)
                for fb in range(nfb):
                    fs = slice(fb * 512, (fb + 1) * 512)
                    p_f = ps_s.tile([128, 512], FP32, tag="s", name="s")
                    for i, (l, off, rows, _) in enumerate(wf_chunk_rows):
                        if l < 3:
                            rhs = h_bufs[l][off:off + rows, fs]
                        else:
                            rhs = h4[off // 128][:rows, fs] if h_dim[3] > 128 \
                                else h4[0][:rows, fs]
                        nc.tensor.matmul(p_f[:], lhsT=wf[i][:, ms], rhs=rhs,
                                         start=(i == 0),
                                         stop=(i == len(wf_chunk_rows) - 1))
                    nc.vector.tensor_reduce(
                        out=red[:, mb * nfb + fb:mb * nfb + fb + 1],
                        in_=p_f[:], axis=mybir.AxisListType.X, op=amax)
            nc.vector.tensor_reduce(
                out=out_sb[:],
                in_=red[:].rearrange("p (m f) -> p m f", f=nfb),
                axis=mybir.AxisListType.X, op=amax)
            nc.sync.dma_start(out_dram.ap(), out_sb[:])

    nc.compile()
    return nc


def _prep_core_inputs(x_c, params, h_dim, in_chan):
    """Host-side input prep for one core: transpose x, split/derive weights."""
    cins = [in_chan] + [h for h in h_dim[:-1]]
    m = {"xT": np.ascontiguousarray(x_c.T)}
    for l in range(4):
        c = cins[l]
        w = params[f"W{l}"]
        m[f"uw{l}"] = np.ascontiguousarray(w[:c])
        m[f"vw{l}"] = np.ascontiguousarray(w[c:] - w[:c])
        m[f"vb{l}"] = params[f"b{l}"][None, :].astype(np.float32)
    wfull = params["Wf"]
    i = 0
    acc = 0
    for l in range(4):
        d = h_dim[l]
        off = 0
        while off < d:
            rows = min(128, d - off)
            m[f"wf{i}"] = np.ascontiguousarray(wfull[acc:acc + rows])
            acc += rows
            off += rows
            i += 1
    return m


_NC_CACHE = {}


def kernel(**inputs):
    x = np.asarray(inputs["x"], dtype=np.float32)
    params = {k_: np.asarray(v, dtype=np.float32) for k_, v in inputs.items()
              if k_ != "x"}

    if "nc" not in _NC_CACHE:
        _NC_CACHE["nc"] = build_program()
    nc = _NC_CACHE["nc"]

    in_maps = [_prep_core_inputs(x[c], params, H_DIM, IN_CHAN)
               for c in range(B)]
    res = bass_utils.run_bass_kernel_spmd(nc, in_maps,
                                          core_ids=list(range(N_CORES)))
    bf = params["bf"]
    out = np.stack([res.results[c]["out"].T.ravel() for c in range(B)])
    return (out + bf[None, :]).astype(np.float32)

